# Initial kernel scaffold
#
"""Multi-head attention Trainium2 kernel (B=4, T=2048, C=1024, H=16).

Sharding: 8 cores = 4 batches x 2 head-groups (8 heads each).
Each core computes, for its (batch b, head set Hc):
  QhT = (Wq[Hc]/sqrt(dk)) @ x_q^T        [512, 2048]  (head dims on partitions)
  KhT =  Wk[Hc]          @ x_k^T         [512, 2048]
  Vh  =  x_v @ Wv[Hc]^T                  [2048, 512]  (+ ones column per head)
  per head: S^T = Kh @ Qh^T  (k on partitions), P = exp(S^T) * mask^T,
            Yaug^T = [Vh|1]^T @ P^T  -> rows 0..63 = Y^T, row 64 = softmax sums
            Y^T normalized by 1/sums -> YaT
  partial = YaT^T @ Wf[:, Hc]^T          [2048, 1024]
Host sums the two head-group partials per batch and adds bf.

All matmuls bf16 with f32 PSUM accumulation. Layouts are chosen so no
on-device transposes are needed (host pre-transposes the inputs).
"""

import numpy as np
import ml_dtypes

import concourse.bass as bass
import concourse.mybir as mybir
import concourse.tile as tile
from concourse import bacc
from concourse.bass_utils import run_bass_kernel_spmd

B, T, C, H = 4, 2048, 1024, 16
DK = C // H            # 64
GH = H // 2            # 8 heads per core
HD = GH * DK           # 512 head-dims per core
P = 128
NQ = 1024              # q-chunk width for attention strips
KS = T // P            # 16 k-strips
NCORES = 8
BF = mybir.dt.bfloat16
F32 = mybir.dt.float32
AF = mybir.ActivationFunctionType

LAST_RESULTS = None
_NC_CACHE = None


def build_bass():
    nc = bacc.Bacc()

    xqT_d = nc.dram_tensor("xqT", [C, T], BF, kind="ExternalInput")
    xkT_d = nc.dram_tensor("xkT", [C, T], BF, kind="ExternalInput")
    xvT_d = nc.dram_tensor("xvT", [C, T], BF, kind="ExternalInput")
    wqT_d = nc.dram_tensor("wqT", [C, HD], BF, kind="ExternalInput")
    wkT_d = nc.dram_tensor("wkT", [C, HD], BF, kind="ExternalInput")
    wvT_d = nc.dram_tensor("wvT", [C, HD], BF, kind="ExternalInput")
    wfT_d = nc.dram_tensor("wfT", [HD, C], BF, kind="ExternalInput")
    bq_d = nc.dram_tensor("bq", [P, HD // P], F32, kind="ExternalInput")
    bk_d = nc.dram_tensor("bk", [P, HD // P], F32, kind="ExternalInput")
    bvt_d = nc.dram_tensor("bvt", [P, HD], F32, kind="ExternalInput")
    maskT_d = nc.dram_tensor("maskT", [T, T], BF, kind="ExternalInput")
    out_d = nc.dram_tensor("out", [T, C], F32, kind="ExternalOutput")

    with tile.TileContext(nc) as tc:
        with (
            tc.tile_pool(name="xs", bufs=10) as xpool,     # x^T strips [128,2048] bf16
            tc.tile_pool(name="ws", bufs=10) as wpool,     # W slices  [128,512]  bf16
            tc.tile_pool(name="wf", bufs=4) as wfpool,     # Wf slices [128,1024] bf16
            tc.tile_pool(name="qk", bufs=8) as qkpool,     # QhT/KhT   [128,2048] bf16
            tc.tile_pool(name="va", bufs=16) as vpool,     # Vaug      [128,520]  bf16
            tc.tile_pool(name="ya", bufs=4) as ypool,      # YaT       [128,2048] bf16
            tc.tile_pool(name="mk", bufs=24) as mpool,     # mask^T    [128,512]  bf16
            tc.tile_pool(name="pp", bufs=8) as ppool,     # P^T pairs [128,1024] bf16
            tc.tile_pool(name="ob", bufs=2) as opool,      # out stage [128,1024] f32
            tc.tile_pool(name="sm", bufs=4) as small,
            tc.tile_pool(name="c1", bufs=1) as const_pool,
            tc.tile_pool(name="psA", bufs=4, space="PSUM") as psA,
        ):
            # constants
            bias_q = const_pool.tile([P, HD // P], F32, tag="bq", name="bq")
            nc.sync.dma_start(out=bias_q[:], in_=bq_d[:])
            bias_k = const_pool.tile([P, HD // P], F32, tag="bk", name="bk")
            nc.sync.dma_start(out=bias_k[:], in_=bk_d[:])
            bvt_sb = const_pool.tile([P, HD], F32, tag="bvt", name="bvt")
            nc.sync.dma_start(out=bvt_sb[:], in_=bvt_d[:])

            wf_sb = []
            for kc in range(HD // P):
                wt = wfpool.tile([P, C], BF, tag="wf", name="wf")
                nc.sync.dma_start(out=wt[:], in_=wfT_d[kc * P:(kc + 1) * P, :])
                wf_sb.append(wt)

            # ---------------- projections ----------------
            # Q^T and K^T: out[mc] = (W^T chunk).T @ x^T  -> [128 hd, T]
            qkT = {}
            for name, xT_d, wT_d, bias in (
                ("q", xqT_d, wqT_d, bias_q),
                ("k", xkT_d, wkT_d, bias_k),
            ):
                xs = []
                ws = []
                for kc in range(C // P):
                    wt = wpool.tile([P, HD], BF, tag="ws", name="ws")
                    nc.sync.dma_start(out=wt[:], in_=wT_d[kc * P:(kc + 1) * P, :])
                    ws.append(wt)
                    xt = xpool.tile([P, T], BF, tag="xs", name="xs")
                    nc.sync.dma_start(out=xt[:], in_=xT_d[kc * P:(kc + 1) * P, :])
                    xs.append(xt)
                tiles = []
                for mc in range(HD // P):
                    dst = qkpool.tile([P, T], BF, tag="qk", name="qk")
                    tiles.append(dst)
                    for half in range(T // NQ):
                        ps = psA.tile([P, NQ], F32, tag="mm", name="mm")
                        for nn in range(NQ // 512):
                            for kc in range(C // P):
                                nc.tensor.matmul(
                                    ps[:, nn * 512:(nn + 1) * 512],
                                    lhsT=ws[kc][:, mc * P:(mc + 1) * P],
                                    rhs=xs[kc][:, half * NQ + nn * 512:
                                               half * NQ + (nn + 1) * 512],
                                    start=(kc == 0),
                                    stop=(kc == C // P - 1),
                                )
                        nc.scalar.activation(
                            dst[:, half * NQ:(half + 1) * NQ],
                            ps[:],
                            AF.Identity,
                            bias=bias[:, mc:mc + 1],
                        )
                qkT[name] = tiles

            # prefetch first q-chunk's mask strips during the projections
            mk0 = []
            for ks in range(KS):
                mt = mpool.tile([P, 512], BF, tag="mk", name="mk")
                nc.sync.dma_start(out=mt[:], in_=maskT_d[ks * P:(ks + 1) * P, 0:512])
                mk0.append(mt)

            # V: natural layout [t, hd] with a ones column per head ([V_h | 1])
            vts = []
            for i in range(KS):
                vt = vpool.tile([P, GH * 65], BF, tag="va", name="va")
                nc.vector.memset(
                    vt.rearrange("p (h e) -> p h e", e=65)[:, :, 64:65], 1.0
                )
                vts.append(vt)
            xs = []
            ws = []
            for kc in range(C // P):
                wt = wpool.tile([P, HD], BF, tag="ws", name="ws")
                nc.sync.dma_start(out=wt[:], in_=wvT_d[kc * P:(kc + 1) * P, :])
                ws.append(wt)
                xt = xpool.tile([P, T], BF, tag="xs", name="xs")
                nc.sync.dma_start(out=xt[:], in_=xvT_d[kc * P:(kc + 1) * P, :])
                xs.append(xt)
            for mc in range(KS):
                ps = psA.tile([P, HD], F32, tag="mm", name="mm")
                for kc in range(C // P):
                    nc.tensor.matmul(
                        ps[:],
                        lhsT=xs[kc][:, mc * P:(mc + 1) * P],
                        rhs=ws[kc][:],
                        start=(kc == 0),
                        stop=(kc == C // P - 1),
                    )
                nc.vector.tensor_add(
                    vts[mc].rearrange("p (h e) -> p h e", e=65)[:, :, 0:64],
                    ps.rearrange("p (h d) -> p h d", d=DK),
                    bvt_sb.rearrange("p (h d) -> p h d", d=DK),
                )

            # ---------------- attention + fc ----------------
            # Head PAIRS (2hp, 2hp+1) live at partition offsets 0/64 of qkT
            # tile hp. Their S^T strips go into one psum tile [128, 2*NQA]
            # as 64-row PE array tiles (tile_position rows 0/64) so the two
            # K=64 matmuls can overlap. One exp covers both. PV accumulates
            # per head into [65, NQA] psum (1 bank), softmax sums in row 64.
            NQA = 512
            yaT = [ypool.tile([P, T], BF, tag="ya", name="ya") for _ in range(HD // P)]

            def emit_fc(mc):
                fps = psA.tile([P, C], F32, tag="mm", name="mm")
                for nn in range(C // 512):
                    for kc in range(HD // P):
                        nc.tensor.matmul(
                            fps[:, nn * 512:(nn + 1) * 512],
                            lhsT=yaT[kc][:, mc * P:(mc + 1) * P],
                            rhs=wf_sb[kc][:, nn * 512:(nn + 1) * 512],
                            start=(kc == 0),
                            stop=(kc == HD // P - 1),
                        )
                ot = opool.tile([P, C], F32, tag="ob", name="ob")
                nc.scalar.activation(ot[:], fps[:], AF.Copy)
                nc.sync.dma_start(out=out_d[mc * P:(mc + 1) * P, :], in_=ot[:])

            for qq in range(T // NQA):
                if qq == 0:
                    mk = mk0
                else:
                    mk = []
                    for ks in range(KS):
                        mt = mpool.tile([P, NQA], BF, tag="mk", name="mk")
                        nc.sync.dma_start(
                            out=mt[:],
                            in_=maskT_d[ks * P:(ks + 1) * P,
                                        qq * NQA:(qq + 1) * NQA],
                        )
                        mk.append(mt)

                for hp in range(GH // 2):
                    qt = qkT["q"][hp]
                    kt = qkT["k"][hp]
                    yp = psA.tile([P, 2 * NQA], F32, tag="mm", name="acc")
                    yps = [yp[:, 0:NQA], yp[:, NQA:2 * NQA]]
                    DLY = 6
                    pts = {}

                    def emit_pv(ks):
                        pt = pts.pop(ks)
                        for hh in range(2):
                            h = 2 * hp + hh
                            nc.tensor.matmul(
                                yps[hh][0:65, :],
                                lhsT=vts[ks][:, h * 65:(h + 1) * 65],
                                rhs=pt[:, hh * NQA:(hh + 1) * NQA],
                                start=(ks == 0),
                                stop=(ks == KS - 1),
                                skip_group_check=True,
                            )

                    for ks in range(KS):
                        sps = psA.tile([P, 2 * NQA], F32, tag="mm", name="mm")
                        for hh in range(2):
                            po = hh * DK
                            nc.tensor.matmul(
                                sps[:, hh * NQA:(hh + 1) * NQA],
                                lhsT=kt[po:po + DK, ks * P:(ks + 1) * P],
                                rhs=qt[po:po + DK,
                                       qq * NQA:(qq + 1) * NQA],
                                start=True,
                                stop=True,
                            )
                        pt = ppool.tile([P, 2 * NQA], BF, tag="pp", name="pp")
                        nc.scalar.activation(pt[:], sps[:], AF.Exp)
                        meng = nc.vector
                        for hh in range(2):
                            meng.tensor_mul(
                                pt[:, hh * NQA:(hh + 1) * NQA],
                                pt[:, hh * NQA:(hh + 1) * NQA],
                                mk[ks][:],
                            )
                        pts[ks] = pt
                        if ks >= DLY:
                            emit_pv(ks - DLY)
                        if ks == 8 and qq > 0:
                            emit_fc((qq - 1) * (NQA // P) + hp)
                    for ks in range(KS - DLY, KS):
                        emit_pv(ks)
                    # normalize: rows 0..63 scaled by 1/row64 (softmax sums).
                    # reciprocal of [1, NQA] on one partition is ~6.5us on DVE,
                    # so reshape to [128, NQA/128] via a DRAM round-trip first.
                    for hh in range(2):
                        h = 2 * hp + hh
                        po = hh * DK
                        srow = small.tile([1, NQA], F32, tag="rc", name="rc")
                        nc.scalar.activation(srow[:], yps[hh][64:65, :], AF.Copy)
                        # partition-scatter [1, NQA] -> [128, NQA/128] so the
                        # reciprocal uses all lanes (one-lane recip is ~6.5us)
                        spread = small.tile([P, NQA // P], F32, tag="sp", name="sp")
                        nc.sync.dma_start(
                            out=spread[:],
                            in_=bass.AP(tensor=srow.tensor, offset=srow.offset,
                                        ap=[[1, 1], [NQA // P, P], [1, NQA // P]]),
                        )
                        spread_r = small.tile([P, NQA // P], F32, tag="sr", name="sr")
                        nc.vector.reciprocal(spread_r[:], spread[:])
                        rrow = small.tile([1, NQA], F32, tag="rr", name="rr")
                        nc.sync.dma_start(
                            out=bass.AP(tensor=rrow.tensor, offset=rrow.offset,
                                        ap=[[1, 1], [NQA // P, P], [1, NQA // P]]),
                            in_=spread_r[:],
                        )
                        rb = small.tile([DK, NQA], F32, tag="rb", name="rb")
                        nc.sync.dma_start(
                            out=rb[:],
                            in_=bass.AP(tensor=rrow.tensor, offset=rrow.offset,
                                        ap=[[1, 1], [0, DK], [1, NQA]]),
                        )
                        nc.vector.tensor_mul(
                            yaT[hp][po:po + DK, qq * NQA:(qq + 1) * NQA],
                            yps[hh][0:64, :],
                            rb[:],
                        )

            # drain the last q-chunk's fc
            for mc in range((T // NQA - 1) * (NQA // P), T // P):
                emit_fc(mc)
    return nc


def shard_inputs(q, k, v, mask, Wq, bq, Wk, bk, Wv, bv, Wf, bf):
    """Build the 8 per-core input maps (host-side prep, numpy only)."""
    bfl = ml_dtypes.bfloat16
    s = 1.0 / np.sqrt(DK)
    q, k, v = (np.asarray(a, np.float32) for a in (q, k, v))
    mask = np.asarray(mask)
    Wq, bq, Wk, bk, Wv, bv, Wf, bf = (
        np.asarray(a, np.float32) for a in (Wq, bq, Wk, bk, Wv, bv, Wf, bf)
    )
    in_maps = []
    for c in range(NCORES):
        b_, g = divmod(c, 2)
        hd = slice(g * HD, (g + 1) * HD)
        im = {
            "xqT": np.ascontiguousarray(q[b_].T.astype(bfl)),
            "xkT": np.ascontiguousarray(k[b_].T.astype(bfl)),
            "xvT": np.ascontiguousarray(v[b_].T.astype(bfl)),
            "wqT": np.ascontiguousarray((Wq[hd, :] * s).T.astype(bfl)),
            "wkT": np.ascontiguousarray(Wk[hd, :].T.astype(bfl)),
            "wvT": np.ascontiguousarray(Wv[hd, :].T.astype(bfl)),
            "wfT": np.ascontiguousarray(Wf[:, hd].T.astype(bfl)),
            "bq": np.ascontiguousarray((bq[hd] * s).reshape(HD // P, P).T),
            "bk": np.ascontiguousarray(bk[hd].reshape(HD // P, P).T),
            "bvt": np.ascontiguousarray(
                np.broadcast_to(bv[hd], (P, HD)).astype(np.float32)
            ),
            "maskT": np.ascontiguousarray(
                (mask[b_] != 0).T.astype(np.float32).astype(bfl)
            ),
        }
        in_maps.append(im)
    return in_maps


def _get_bass():
    global _NC_CACHE
    if _NC_CACHE is None:
        nc = build_bass()
        nc.finalize()
        _NC_CACHE = nc
    return _NC_CACHE


def kernel(q, k, v, mask, Wq, bq, Wk, bk, Wv, bv, Wf, bf):
    global LAST_RESULTS
    nc = _get_bass()
    in_maps = shard_inputs(q, k, v, mask, Wq, bq, Wk, bk, Wv, bv, Wf, bf)
    res = run_bass_kernel_spmd(nc, in_maps, core_ids=list(range(NCORES)))
    LAST_RESULTS = res
    bf32 = np.asarray(bf, np.float32)
    out = np.empty((B, T, C), np.float32)
    for b_ in range(B):
        out[b_] = (
            res.results[2 * b_]["out"]
            + res.results[2 * b_ + 1]["out"]
            + bf32[None, :]
        )
    return out



# revision 1
# speedup vs baseline: 1.0843x; 1.0843x over previous
"""Multi-head attention Trainium2 kernel (B=4, T=2048, C=1024, H=16).

Sharding: 8 cores = 4 batches x 2 head-groups (8 heads each).
Each core computes, for its (batch b, head set Hc):
  QhT = (Wq[Hc]/sqrt(dk)) @ x_q^T        [512, 2048]  (head dims on partitions)
  KhT =  Wk[Hc]          @ x_k^T         [512, 2048]
  Vh  =  x_v @ Wv[Hc]^T                  [2048, 512]  (+ ones column per head)
  per head: S^T = Kh @ Qh^T  (k on partitions), P = exp(S^T) * mask^T,
            Yaug^T = [Vh|1]^T @ P^T  -> rows 0..63 = Y^T, row 64 = softmax sums
            Y^T normalized by 1/sums -> YaT
  partial = YaT^T @ Wf[:, Hc]^T          [2048, 1024]
Host sums the two head-group partials per batch and adds bf.

All matmuls bf16 with f32 PSUM accumulation. Layouts are chosen so no
on-device transposes are needed (host pre-transposes the inputs).
"""

import numpy as np
import ml_dtypes

import concourse.bass as bass
import concourse.mybir as mybir
import concourse.tile as tile
from concourse import bacc
from concourse.bass_utils import run_bass_kernel_spmd

B, T, C, H = 4, 2048, 1024, 16
DK = C // H            # 64
GH = H // 2            # 8 heads per core
HD = GH * DK           # 512 head-dims per core
P = 128
NQ = 1024              # q-chunk width for attention strips
KS = T // P            # 16 k-strips
NCORES = 8
BF = mybir.dt.bfloat16
F32 = mybir.dt.float32
AF = mybir.ActivationFunctionType

LAST_RESULTS = None
_NC_CACHE = None


def build_bass():
    nc = bacc.Bacc()

    xqT_d = nc.dram_tensor("xqT", [C, T], BF, kind="ExternalInput")
    xkT_d = nc.dram_tensor("xkT", [C, T], BF, kind="ExternalInput")
    xvT_d = nc.dram_tensor("xvT", [C, T], BF, kind="ExternalInput")
    wqT_d = nc.dram_tensor("wqT", [C, HD], BF, kind="ExternalInput")
    wkT_d = nc.dram_tensor("wkT", [C, HD], BF, kind="ExternalInput")
    wvT_d = nc.dram_tensor("wvT", [C, HD], BF, kind="ExternalInput")
    wfT_d = nc.dram_tensor("wfT", [HD, C], BF, kind="ExternalInput")
    bq_d = nc.dram_tensor("bq", [P, HD // P], F32, kind="ExternalInput")
    bk_d = nc.dram_tensor("bk", [P, HD // P], F32, kind="ExternalInput")
    bvt_d = nc.dram_tensor("bvt", [P, HD], F32, kind="ExternalInput")
    maskT_d = nc.dram_tensor("maskT", [T, T], BF, kind="ExternalInput")
    out_d = nc.dram_tensor("out", [T, C], F32, kind="ExternalOutput")

    with tile.TileContext(nc) as tc:
        with (
            tc.tile_pool(name="xs", bufs=10) as xpool,     # x^T strips [128,2048] bf16
            tc.tile_pool(name="ws", bufs=10) as wpool,     # W slices  [128,512]  bf16
            tc.tile_pool(name="wf", bufs=4) as wfpool,     # Wf slices [128,1024] bf16
            tc.tile_pool(name="qk", bufs=8) as qkpool,     # QhT/KhT   [128,2048] bf16
            tc.tile_pool(name="va", bufs=16) as vpool,     # Vaug      [128,520]  bf16
            tc.tile_pool(name="ya", bufs=4) as ypool,      # YaT       [128,2048] bf16
            tc.tile_pool(name="mk", bufs=24) as mpool,     # mask^T    [128,512]  bf16
            tc.tile_pool(name="pp", bufs=8) as ppool,     # P^T pairs [128,1024] bf16
            tc.tile_pool(name="ob", bufs=2) as opool,      # out stage [128,1024] f32
            tc.tile_pool(name="sm", bufs=4) as small,
            tc.tile_pool(name="c1", bufs=1) as const_pool,
            tc.tile_pool(name="psA", bufs=4, space="PSUM") as psA,
        ):
            # constants
            bias_q = const_pool.tile([P, HD // P], F32, tag="bq", name="bq")
            nc.sync.dma_start(out=bias_q[:], in_=bq_d[:])
            bias_k = const_pool.tile([P, HD // P], F32, tag="bk", name="bk")
            nc.sync.dma_start(out=bias_k[:], in_=bk_d[:])
            bvt_sb = const_pool.tile([P, HD], F32, tag="bvt", name="bvt")
            nc.sync.dma_start(out=bvt_sb[:], in_=bvt_d[:])

            wf_sb = []
            for kc in range(HD // P):
                wt = wfpool.tile([P, C], BF, tag="wf", name="wf")
                nc.sync.dma_start(out=wt[:], in_=wfT_d[kc * P:(kc + 1) * P, :])
                wf_sb.append(wt)

            # ---------------- projections ----------------
            # Q^T and K^T: out[mc] = (W^T chunk).T @ x^T  -> [128 hd, T]
            qkT = {}
            for name, xT_d, wT_d, bias in (
                ("q", xqT_d, wqT_d, bias_q),
                ("k", xkT_d, wkT_d, bias_k),
            ):
                xs = []
                ws = []
                for kc in range(C // P):
                    wt = wpool.tile([P, HD], BF, tag="ws", name="ws")
                    nc.sync.dma_start(out=wt[:], in_=wT_d[kc * P:(kc + 1) * P, :])
                    ws.append(wt)
                    xt = xpool.tile([P, T], BF, tag="xs", name="xs")
                    nc.sync.dma_start(out=xt[:], in_=xT_d[kc * P:(kc + 1) * P, :])
                    xs.append(xt)
                tiles = []
                for mc in range(HD // P):
                    dst = qkpool.tile([P, T], BF, tag="qk", name="qk")
                    tiles.append(dst)
                    for half in range(T // NQ):
                        ps = psA.tile([P, NQ], F32, tag="mm", name="mm")
                        for nn in range(NQ // 512):
                            for kc in range(C // P):
                                nc.tensor.matmul(
                                    ps[:, nn * 512:(nn + 1) * 512],
                                    lhsT=ws[kc][:, mc * P:(mc + 1) * P],
                                    rhs=xs[kc][:, half * NQ + nn * 512:
                                               half * NQ + (nn + 1) * 512],
                                    start=(kc == 0),
                                    stop=(kc == C // P - 1),
                                )
                        nc.scalar.activation(
                            dst[:, half * NQ:(half + 1) * NQ],
                            ps[:],
                            AF.Identity,
                            bias=bias[:, mc:mc + 1],
                        )
                qkT[name] = tiles

            # prefetch first q-chunk's mask strips during the projections
            mk0 = []
            for ks in range(KS):
                mt = mpool.tile([P, 512], BF, tag="mk", name="mk")
                nc.sync.dma_start(out=mt[:], in_=maskT_d[ks * P:(ks + 1) * P, 0:512])
                mk0.append(mt)

            # V: natural layout [t, hd] with a ones column per head ([V_h | 1])
            vts = []
            for i in range(KS):
                vt = vpool.tile([P, GH * 65], BF, tag="va", name="va")
                nc.vector.memset(
                    vt.rearrange("p (h e) -> p h e", e=65)[:, :, 64:65], 1.0
                )
                vts.append(vt)
            xs = []
            ws = []
            for kc in range(C // P):
                wt = wpool.tile([P, HD], BF, tag="ws", name="ws")
                nc.sync.dma_start(out=wt[:], in_=wvT_d[kc * P:(kc + 1) * P, :])
                ws.append(wt)
                xt = xpool.tile([P, T], BF, tag="xs", name="xs")
                nc.sync.dma_start(out=xt[:], in_=xvT_d[kc * P:(kc + 1) * P, :])
                xs.append(xt)
            for mc in range(KS):
                ps = psA.tile([P, HD], F32, tag="mm", name="mm")
                for kc in range(C // P):
                    nc.tensor.matmul(
                        ps[:],
                        lhsT=xs[kc][:, mc * P:(mc + 1) * P],
                        rhs=ws[kc][:],
                        start=(kc == 0),
                        stop=(kc == C // P - 1),
                    )
                nc.vector.tensor_add(
                    vts[mc].rearrange("p (h e) -> p h e", e=65)[:, :, 0:64],
                    ps.rearrange("p (h d) -> p h d", d=DK),
                    bvt_sb.rearrange("p (h d) -> p h d", d=DK),
                )

            # ---------------- attention + fc ----------------
            # Head PAIRS (2hp, 2hp+1) live at partition offsets 0/64 of qkT
            # tile hp. Their S^T strips go into one psum tile [128, 2*NQA]
            # as 64-row PE array tiles (tile_position rows 0/64) so the two
            # K=64 matmuls can overlap. One exp covers both. PV accumulates
            # per head into [65, NQA] psum (1 bank), softmax sums in row 64.
            NQA = 512
            yaT = [ypool.tile([P, T], BF, tag="ya", name="ya") for _ in range(HD // P)]

            def emit_fc(mc):
                fps = psA.tile([P, C], F32, tag="mm", name="mm")
                for nn in range(C // 512):
                    for kc in range(HD // P):
                        nc.tensor.matmul(
                            fps[:, nn * 512:(nn + 1) * 512],
                            lhsT=yaT[kc][:, mc * P:(mc + 1) * P],
                            rhs=wf_sb[kc][:, nn * 512:(nn + 1) * 512],
                            start=(kc == 0),
                            stop=(kc == HD // P - 1),
                        )
                ot = opool.tile([P, C], F32, tag="ob", name="ob")
                nc.scalar.activation(ot[:], fps[:], AF.Copy)
                nc.sync.dma_start(out=out_d[mc * P:(mc + 1) * P, :], in_=ot[:])

            for qq in range(T // NQA):
                if qq == 0:
                    mk = mk0
                else:
                    mk = []
                    for ks in range(KS):
                        mt = mpool.tile([P, NQA], BF, tag="mk", name="mk")
                        nc.sync.dma_start(
                            out=mt[:],
                            in_=maskT_d[ks * P:(ks + 1) * P,
                                        qq * NQA:(qq + 1) * NQA],
                        )
                        mk.append(mt)

                for hp in range(GH // 2):
                    qt = qkT["q"][hp]
                    kt = qkT["k"][hp]
                    yp = psA.tile([P, 2 * NQA], F32, tag="mm", name="acc")
                    yps = [yp[:, 0:NQA], yp[:, NQA:2 * NQA]]
                    DLY = 6
                    pts = {}

                    def emit_pv(ks):
                        pt = pts.pop(ks)
                        for hh in range(2):
                            h = 2 * hp + hh
                            nc.tensor.matmul(
                                yps[hh][0:65, :],
                                lhsT=vts[ks][:, h * 65:(h + 1) * 65],
                                rhs=pt[:, hh * NQA:(hh + 1) * NQA],
                                start=(ks == 0),
                                stop=(ks == KS - 1),
                                skip_group_check=True,
                            )

                    for ks in range(KS):
                        sps = psA.tile([P, 2 * NQA], F32, tag="mm", name="mm")
                        for hh in range(2):
                            po = hh * DK
                            nc.tensor.matmul(
                                sps[:, hh * NQA:(hh + 1) * NQA],
                                lhsT=kt[po:po + DK, ks * P:(ks + 1) * P],
                                rhs=qt[po:po + DK,
                                       qq * NQA:(qq + 1) * NQA],
                                start=True,
                                stop=True,
                            )
                        pt = ppool.tile([P, 2 * NQA], BF, tag="pp", name="pp")
                        nc.scalar.activation(pt[:], sps[:], AF.Exp)
                        meng = nc.vector
                        for hh in range(2):
                            meng.tensor_mul(
                                pt[:, hh * NQA:(hh + 1) * NQA],
                                pt[:, hh * NQA:(hh + 1) * NQA],
                                mk[ks][:],
                            )
                        pts[ks] = pt
                        if ks >= DLY:
                            emit_pv(ks - DLY)
                        if ks == 8 and qq > 0:
                            emit_fc((qq - 1) * (NQA // P) + hp)
                    for ks in range(KS - DLY, KS):
                        emit_pv(ks)
                    # normalize: rows 0..63 scaled by 1/row64 (softmax sums).
                    # reciprocal of [1, NQA] on one partition is ~6.5us on DVE,
                    # so reshape to [128, NQA/128] via a DRAM round-trip first.
                    for hh in range(2):
                        h = 2 * hp + hh
                        po = hh * DK
                        srow = small.tile([1, NQA], F32, tag="rc", name="rc")
                        nc.scalar.activation(srow[:], yps[hh][64:65, :], AF.Copy)
                        # partition-scatter [1, NQA] -> [128, NQA/128] so the
                        # reciprocal uses all lanes (one-lane recip is ~6.5us)
                        spread = small.tile([P, NQA // P], F32, tag="sp", name="sp")
                        nc.sync.dma_start(
                            out=spread[:],
                            in_=bass.AP(tensor=srow.tensor, offset=srow.offset,
                                        ap=[[1, 1], [NQA // P, P], [1, NQA // P]]),
                        )
                        spread_r = small.tile([P, NQA // P], F32, tag="sr", name="sr")
                        nc.vector.reciprocal(spread_r[:], spread[:])
                        rrow = small.tile([1, NQA], F32, tag="rr", name="rr")
                        nc.sync.dma_start(
                            out=bass.AP(tensor=rrow.tensor, offset=rrow.offset,
                                        ap=[[1, 1], [NQA // P, P], [1, NQA // P]]),
                            in_=spread_r[:],
                        )
                        rb = small.tile([DK, NQA], F32, tag="rb", name="rb")
                        nc.sync.dma_start(
                            out=rb[:],
                            in_=bass.AP(tensor=rrow.tensor, offset=rrow.offset,
                                        ap=[[1, 1], [0, DK], [1, NQA]]),
                        )
                        nc.vector.tensor_mul(
                            yaT[hp][po:po + DK, qq * NQA:(qq + 1) * NQA],
                            yps[hh][0:64, :],
                            rb[:],
                        )

            # drain the last q-chunk's fc
            for mc in range((T // NQA - 1) * (NQA // P), T // P):
                emit_fc(mc)
    return nc


def shard_inputs(q, k, v, mask, Wq, bq, Wk, bk, Wv, bv, Wf, bf):
    """Build the 8 per-core input maps (host-side prep, numpy only)."""
    bfl = ml_dtypes.bfloat16
    s = 1.0 / np.sqrt(DK)
    q, k, v = (np.asarray(a, np.float32) for a in (q, k, v))
    mask = np.asarray(mask)
    Wq, bq, Wk, bk, Wv, bv, Wf, bf = (
        np.asarray(a, np.float32) for a in (Wq, bq, Wk, bk, Wv, bv, Wf, bf)
    )
    in_maps = []
    for c in range(NCORES):
        b_, g = divmod(c, 2)
        hd = slice(g * HD, (g + 1) * HD)
        im = {
            "xqT": np.ascontiguousarray(q[b_].T.astype(bfl)),
            "xkT": np.ascontiguousarray(k[b_].T.astype(bfl)),
            "xvT": np.ascontiguousarray(v[b_].T.astype(bfl)),
            "wqT": np.ascontiguousarray((Wq[hd, :] * s).T.astype(bfl)),
            "wkT": np.ascontiguousarray(Wk[hd, :].T.astype(bfl)),
            "wvT": np.ascontiguousarray(Wv[hd, :].T.astype(bfl)),
            "wfT": np.ascontiguousarray(Wf[:, hd].T.astype(bfl)),
            "bq": np.ascontiguousarray((bq[hd] * s).reshape(HD // P, P).T),
            "bk": np.ascontiguousarray(bk[hd].reshape(HD // P, P).T),
            "bvt": np.ascontiguousarray(
                np.broadcast_to(bv[hd], (P, HD)).astype(np.float32)
            ),
            "maskT": np.ascontiguousarray(
                (mask[b_] != 0).T.astype(np.float32).astype(bfl)
            ),
        }
        in_maps.append(im)
    return in_maps


def _get_bass():
    global _NC_CACHE
    if _NC_CACHE is None:
        nc = build_bass()
        nc.finalize()
        _NC_CACHE = nc
    return _NC_CACHE


def kernel(q, k, v, mask, Wq, bq, Wk, bk, Wv, bv, Wf, bf):
    global LAST_RESULTS
    nc = _get_bass()
    in_maps = shard_inputs(q, k, v, mask, Wq, bq, Wk, bk, Wv, bv, Wf, bf)
    res = run_bass_kernel_spmd(nc, in_maps, core_ids=list(range(NCORES)))
    LAST_RESULTS = res
    bf32 = np.asarray(bf, np.float32)
    out = np.empty((B, T, C), np.float32)
    for b_ in range(B):
        out[b_] = (
            res.results[2 * b_]["out"]
            + res.results[2 * b_ + 1]["out"]
            + bf32[None, :]
        )
    return out

